# revision 2
# baseline (speedup 1.0000x reference)
"""Trainium2 Bass kernel for single-step (decode) multi-head attention.

Module: y = o_proj(SDPA(q, K_cache<-k, V_cache<-v)) for B=16, S=1, D=2048,
H=16 heads, head_dim=128, KV cache length 4096, with the new k/v written at
`position` before attention.

Sharding: tensor-parallel over heads. 8 cores x 2 heads each. Each core gets
its slice of Wq/Wk/Wv rows (256 of 2048), Wo columns, and the K/V cache for
its 2 heads; it computes q/k/v projections, attention over the cache, and a
partial o_proj per head. The host sums the 16 per-head partial outputs.

DMA plan (the kernel is HBM-bandwidth bound on the cache stream, ~67 MB/core
in bf16): K and V are packed on the host into one DRAM tensor of 16 chunks x
(K-half + V-half), 2 (head,batch) pairs per chunk, so each dma_start moves
2 MB with 16 KB contiguous per SBUF partition. The K stream issues on the
sync (SP) HWDGE ring and the V stream on the scalar (ACT) ring, weights on
the gpsimd (SWDGE) ring, so the 16 SDMA engines round-robin across queues
and per-DMA boundary bubbles are hidden.

Per-pair critical path is two cross-engine hops: PE score matmuls -> ACT exp
-> PE attn@V matmuls. The cache update at `position` never touches the
per-pair path: the stale K contribution is zeroed inside the exp via a
per-partition bias tile (-1e30 at partition pos%128, applied to kv-chunk
pos//128 only), and the new token's attention weight anew = exp(scale *
sum(q*k_new)) plus its V contribution anew*v_new are computed once, batched,
right after the projections. The per-head epilogue only sums partials
(+anew), broadcasts 1/Z, applies it, and runs o_proj; head 0's epilogue and
output DMA hide under the stream at pair 15.

Precision: cache matmuls and projections in bf16 (PE native rate) with fp32
PSUM accumulation; softmax sums, normalization and the new-token V term in
fp32. Measured vs fp32 reference: rel err ~5e-3.
"""

import sys

for _p in ("/opt/trn_rl_repo", "/root/.axon_site/_ro/trn_rl_repo"):
    if _p not in sys.path:
        sys.path.append(_p)

import ml_dtypes
import numpy as np

import concourse.bacc as bacc
import concourse.mybir as mybir
import concourse.tile as tile
from concourse.bass_utils import run_bass_kernel_spmd

F32 = mybir.dt.float32
BF16 = mybir.dt.bfloat16

B = 16          # batch
D = 2048        # model dim
H_TOT = 16      # total heads
HD = 128        # head dim
KV = 4096       # cache length
KVC = KV // 128  # 32 kv chunks of 128
N_CORES = 8
H_LOC = H_TOT // N_CORES       # 2 heads per core
PAIRS = H_LOC * B              # 32 (b,h) pairs per core
HS = H_LOC * HD                # 256-channel slice per core
DC = D // 128                  # 16 contraction chunks for projections

CH = 2                         # pairs per cache chunk (DMA granularity)
NCH = PAIRS // CH              # 16 chunks
CBUF = 3                       # cache chunks in flight per stream

# Matches reference: scale = 1.0 / np.sqrt(head_dim).astype(np.float32)
SCALE = float(1.0 / np.sqrt(float(HD)).astype(np.float32))

CDT = BF16
LAST_RESULT = None  # BassKernelResults of the most recent run (for profiling)


def build_kernel(position):
    """Trace the per-core Bass kernel. `position` is baked in as a constant."""
    pc, pi = position // 128, position % 128
    assert 0 <= position < KV

    nc = bacc.Bacc("TRN2", target_bir_lowering=False, debug=False)

    xT = nc.dram_tensor("xT", [128, DC, B], CDT, kind="ExternalInput").ap()
    wqT = nc.dram_tensor("wqT", [128, DC, HS], CDT, kind="ExternalInput").ap()
    wkT = nc.dram_tensor("wkT", [128, DC, HS], CDT, kind="ExternalInput").ap()
    wvT = nc.dram_tensor("wvT", [128, DC, HS], CDT, kind="ExternalInput").ap()
    woT = nc.dram_tensor("woT", [128, H_LOC, D], CDT, kind="ExternalInput").ap()
    cache = nc.dram_tensor(
        "cache", [128, NCH, 2, CH * KV], CDT, kind="ExternalInput"
    ).ap()
    yT = nc.dram_tensor("yT", [128, H_LOC, DC, B], F32, kind="ExternalOutput").ap()

    with tile.TileContext(nc) as tc:
        with (
            tc.tile_pool(name="wpool", bufs=1) as wpool,
            tc.tile_pool(name="spool", bufs=1) as spool,
            tc.tile_pool(name="kpool", bufs=CBUF) as kpool,
            tc.tile_pool(name="vpool", bufs=CBUF) as vpool,
            tc.tile_pool(name="ps_sc", bufs=3, space="PSUM") as ps_sc,
            tc.tile_pool(name="ps_one", bufs=1, space="PSUM") as ps_one,
        ):
            # ---- weights & x on the SWDGE (gpsimd) queue; cache streams on
            # the two HWDGE rings so the SDMA engines round-robin between
            # them at packet granularity ----
            xT_sb = wpool.tile([128, DC, B], CDT)
            nc.gpsimd.dma_start(xT_sb[:], xT)
            wq_sb = wpool.tile([128, DC, HS], CDT)
            nc.gpsimd.dma_start(wq_sb[:], wqT)
            wk_sb = wpool.tile([128, DC, HS], CDT)
            nc.gpsimd.dma_start(wk_sb[:], wkT)
            wv_sb = wpool.tile([128, DC, HS], CDT)
            nc.gpsimd.dma_start(wv_sb[:], wvT)
            wo_sb = wpool.tile([128, H_LOC, D], CDT)
            nc.gpsimd.dma_start(wo_sb[:], woT)

            # ---- cache chunk stream: K on sync ring, V on scalar ring ----
            kts, vts = {}, {}

            def issue_chunk(c):
                kt = kpool.tile([128, CH * KV], CDT, tag="kt", name="kt")
                nc.sync.dma_start(kt[:], cache[:, c, 0])
                kts[c] = kt
                vt = vpool.tile([128, CH * KV], CDT, tag="vt", name="vt")
                nc.scalar.dma_start(vt[:], cache[:, c, 1])
                vts[c] = vt

            issue_chunk(0)
            issue_chunk(1)

            # ---- constants ----
            ones_col = spool.tile([128, 1], F32)
            nc.vector.memset(ones_col[:], 1.0)
            ones_row = spool.tile([1, 128], F32)
            nc.vector.memset(ones_row[:], 1.0)
            # mbias: -1e30 at partition pi, 0 elsewhere (zeroes the stale K
            # contribution inside the exp of kv-chunk pc)
            negc = spool.tile([128, 1], F32)
            nc.vector.memset(negc[:], -1e30)
            mbias = spool.tile([128, 1], F32)
            nc.gpsimd.affine_select(
                mbias[:], negc[:], pattern=[[0, 1]],
                compare_op=mybir.AluOpType.is_equal, fill=0.0,
                base=-pi, channel_multiplier=1,
            )

            # ---- q/k/v projections -> (128 hd, 32 pair) columns ----
            qT_sb = spool.tile([128, PAIRS], CDT)
            kn_sb = spool.tile([128, PAIRS], CDT)
            vn_sb = spool.tile([128, PAIRS], F32)  # new-v term applied in fp32
            for w_sb, out_sb, ptag in (
                (wq_sb, qT_sb, "pj_a"),
                (wk_sb, kn_sb, "pj_b"),
                (wv_sb, vn_sb, "pj_a"),
            ):
                pj = ps_one.tile([128, PAIRS], F32, tag=ptag, name="pj")
                for h in range(H_LOC):
                    for c in range(DC):
                        nc.tensor.matmul(
                            pj[:, 16 * h : 16 * (h + 1)],
                            w_sb[:, c, 128 * h : 128 * (h + 1)],
                            xT_sb[:, c, :],
                            start=(c == 0),
                            stop=(c == DC - 1),
                        )
                nc.vector.tensor_copy(out_sb[:], pj[:])

            # ---- batched new-token terms (off the per-pair path):
            # anew = exp(scale * colsum(q .* k_new));  t1 = anew_bc .* v_new ----
            qkn = spool.tile([128, PAIRS], F32)
            nc.vector.tensor_tensor(qkn[:], qT_sb[:], kn_sb[:], mybir.AluOpType.mult)
            snew_ps = ps_one.tile([1, PAIRS], F32, tag="pj_b", name="snew")
            nc.tensor.matmul(snew_ps[:], ones_col[:], qkn[:], start=True, stop=True)
            anew_sb = spool.tile([1, PAIRS], F32)
            nc.scalar.activation(
                anew_sb[:], snew_ps[:], mybir.ActivationFunctionType.Exp, scale=SCALE
            )
            ab_ps = ps_one.tile([128, PAIRS], F32, tag="pj_b", name="ab_ps")
            nc.tensor.matmul(ab_ps[:], ones_row[:], anew_sb[:], start=True, stop=True)
            anew_bc = spool.tile([128, PAIRS], F32)
            nc.scalar.copy(anew_bc[:], ab_ps[:])
            t1_sb = spool.tile([128, PAIRS], F32)
            nc.vector.tensor_tensor(
                t1_sb[:], anew_bc[:], vn_sb[:], mybir.AluOpType.mult
            )

            # ---- per-pair state ----
            attn_sb = spool.tile([128, PAIRS * KVC], CDT)
            pA = spool.tile([128, PAIRS], F32)   # exp row-sums, kv chunks < pc
            pB = spool.tile([128, PAIRS], F32)   # exp row-sums, masked chunk pc
            pC = spool.tile([128, PAIRS], F32)   # exp row-sums, kv chunks > pc
            outU = ps_one.tile([128, PAIRS], F32, tag="outU")
            attout = spool.tile([128, PAIRS], CDT)
            yt_ps = [
                ps_one.tile([128, DC, B], F32, tag="yt0", name="yt0"),
                ps_one.tile([128, DC, B], F32, tag="yt1", name="yt1"),
            ]

            def front(p):
                c, i = p // CH, p % CH
                kt = kts[c]
                sc = ps_sc.tile([128, KVC], F32, tag="sc", name="sc")
                for j in range(KVC):
                    nc.tensor.matmul(
                        sc[:, j : j + 1],
                        kt[:, KV * i + 128 * j : KV * i + 128 * (j + 1)],
                        qT_sb[:, p : p + 1],
                        start=True,
                        stop=True,
                    )
                ab = attn_sb[:, KVC * p : KVC * (p + 1)]
                if pc > 0:
                    nc.scalar.activation(
                        ab[:, 0:pc], sc[:, 0:pc],
                        mybir.ActivationFunctionType.Exp,
                        scale=SCALE, accum_out=pA[:, p : p + 1],
                    )
                nc.scalar.activation(
                    ab[:, pc : pc + 1], sc[:, pc : pc + 1],
                    mybir.ActivationFunctionType.Exp,
                    scale=SCALE, bias=mbias[:], accum_out=pB[:, p : p + 1],
                )
                if pc < KVC - 1:
                    nc.scalar.activation(
                        ab[:, pc + 1 :], sc[:, pc + 1 :],
                        mybir.ActivationFunctionType.Exp,
                        scale=SCALE, accum_out=pC[:, p : p + 1],
                    )

            def back(p):
                c, i = p // CH, p % CH
                vt = vts[c]
                ab = attn_sb[:, KVC * p : KVC * (p + 1)]
                for j in range(KVC):
                    nc.tensor.matmul(
                        outU[:, p : p + 1],
                        vt[:, KV * i + 128 * j : KV * i + 128 * (j + 1)],
                        ab[:, j : j + 1],
                        start=(j == 0),
                        stop=(j == KVC - 1),
                    )

            def epi(h):
                cs = slice(16 * h, 16 * (h + 1))
                # Z per pair: sum of exp row-sums (+ the new token's weight)
                es = ps_one.tile([1, 16], F32, tag="pj_a", name="es")
                nc.tensor.matmul(es[:], ones_col[:], pA[:, cs], start=True, stop=False)
                if pc < KVC - 1:
                    nc.tensor.matmul(
                        es[:], ones_col[:], pC[:, cs], start=False, stop=False
                    )
                nc.tensor.matmul(es[:], ones_col[:], pB[:, cs], start=False, stop=False)
                nc.tensor.matmul(
                    es[:], ones_row[:, :1], anew_sb[:, cs], start=False, stop=True
                )
                recip_h = spool.tile([1, 16], F32, tag=f"recip{h}", name="recip_h")
                nc.vector.reciprocal(recip_h[:], es[:])
                rb = ps_one.tile([128, 16], F32, tag="pj_b", name="rb")
                nc.tensor.matmul(rb[:], ones_row[:], recip_h[:], start=True, stop=True)
                recip_bc = spool.tile([128, 16], F32, tag=f"rbc{h}", name="recip_bc")
                nc.scalar.copy(recip_bc[:], rb[:])
                t2 = spool.tile([128, 16], F32, tag=f"t2{h}", name="t2")
                nc.vector.tensor_tensor(
                    t2[:], outU[:, cs], t1_sb[:, cs], mybir.AluOpType.add
                )
                nc.vector.tensor_tensor(
                    attout[:, cs], t2[:], recip_bc[:], mybir.AluOpType.mult
                )
                for dc in range(DC):
                    nc.tensor.matmul(
                        yt_ps[h][:, dc, :],
                        wo_sb[:, h, 128 * dc : 128 * (dc + 1)],
                        attout[:, cs],
                        start=True,
                        stop=True,
                    )
                yt_sb = spool.tile([128, DC, B], F32, tag=f"yt_sb{h}", name="yt_sb")
                nc.vector.tensor_copy(yt_sb[:], yt_ps[h][:])
                nc.sync.dma_start(yT[:, h], yt_sb[:])

            # software-pipelined: chunk c's fronts run while chunk c-1's backs
            # consume, DMA for chunk c+2 issues ahead (CBUF=3 in flight)
            for c in range(NCH):
                if c + 2 < NCH:
                    issue_chunk(c + 2)
                front(CH * c)
                front(CH * c + 1)
                if c > 0:
                    back(CH * (c - 1))
                    back(CH * (c - 1) + 1)
                if c == 8:
                    epi(0)  # pairs 0-15 (head 0) all finished at c==8
            back(PAIRS - 2)
            back(PAIRS - 1)
            epi(H_LOC - 1)

    nc.compile()
    return nc


def shard_inputs(x, Wq, Wk, Wv, Wo, k_cache, v_cache):
    """Build per-core input maps (head-sharded)."""
    cdt = ml_dtypes.bfloat16

    def sb_layout(a2d, inner):
        # (K*128, inner-layout...) -> (128, K, ...) contiguous per partition
        d0 = a2d.shape[0]
        return np.ascontiguousarray(
            a2d.reshape(d0 // 128, 128, a2d.shape[1]).transpose(1, 0, 2)
        ).astype(cdt)

    x2 = np.ascontiguousarray(np.asarray(x, dtype=np.float32).reshape(B, D))
    xT_full = sb_layout(np.ascontiguousarray(x2.T), B)        # (128, DC, B)

    # K: (hd, H, B, KV) ; V: (kv%128, H, B, KVC, hd) partition-swizzled
    kT_all = np.asarray(k_cache, dtype=np.float32).transpose(3, 1, 0, 2).astype(cdt)
    v_all = (
        np.asarray(v_cache, dtype=np.float32)
        .reshape(B, H_TOT, KVC, 128, HD)
        .transpose(3, 1, 0, 2, 4)
        .astype(cdt)
    )
    Wq = np.asarray(Wq, dtype=np.float32)
    Wk = np.asarray(Wk, dtype=np.float32)
    Wv = np.asarray(Wv, dtype=np.float32)
    Wo = np.asarray(Wo, dtype=np.float32)

    in_maps = []
    for c in range(N_CORES):
        r0, r1 = HS * c, HS * (c + 1)
        h0, h1 = H_LOC * c, H_LOC * (c + 1)
        ktr = kT_all[:, h0:h1].reshape(128, NCH, CH * KV)
        vtr = v_all[:, h0:h1].reshape(128, NCH, CH * KV)
        cache = np.ascontiguousarray(np.stack([ktr, vtr], axis=2))
        in_maps.append(
            {
                "xT": xT_full,
                "wqT": sb_layout(Wq[r0:r1].T, HS),
                "wkT": sb_layout(Wk[r0:r1].T, HS),
                "wvT": sb_layout(Wv[r0:r1].T, HS),
                "woT": sb_layout(Wo[:, r0:r1].T, D),
                "cache": cache,
            }
        )
    return in_maps


_NC_CACHE = {}


def kernel(x, Wq, Wk, Wv, Wo, k_cache, v_cache, position):
    global LAST_RESULT
    pos = int(position)
    nc = _NC_CACHE.get(pos)
    if nc is None:
        nc = _NC_CACHE[pos] = build_kernel(pos)
    in_maps = shard_inputs(x, Wq, Wk, Wv, Wo, k_cache, v_cache)
    res = run_bass_kernel_spmd(nc, in_maps, core_ids=list(range(N_CORES)))
    LAST_RESULT = res
    out = np.zeros((128, DC, B), dtype=np.float32)
    for c in range(N_CORES):
        out += res.results[c]["yT"].sum(axis=1)
    y2 = out.transpose(1, 0, 2).reshape(D, B)
    return np.ascontiguousarray(y2.T).reshape(B, 1, D)


# revision 5
# speedup vs baseline: 1.1136x; 1.1136x over previous
"""Trainium2 Bass kernel for single-step (decode) multi-head attention.

Module: y = o_proj(SDPA(q, K_cache<-k, V_cache<-v)) for B=16, S=1, D=2048,
H=16 heads, head_dim=128, KV cache length 4096, with the new k/v written at
`position` before attention.

Sharding: tensor-parallel over heads. 8 cores x 2 heads each. Each core gets
its slice of Wq/Wk/Wv rows (256 of 2048), Wo columns, and the K/V cache for
its 2 heads; it computes q/k/v projections, attention over the cache, and a
partial o_proj per head. The host sums the 16 per-head partial outputs.

DMA plan (the kernel is HBM-bandwidth bound on the cache stream, ~67 MB/core
in bf16): K and V are packed on the host into one DRAM tensor of 16 chunks x
(K-half + V-half), 2 (head,batch) pairs per chunk, so each dma_start moves
2 MB with 16 KB contiguous per SBUF partition. The K stream issues on the
sync (SP) HWDGE ring and the V stream on the scalar (ACT) ring, weights on
the gpsimd (SWDGE) ring, so the 16 SDMA engines round-robin across queues
and per-DMA boundary bubbles are hidden.

Per-pair critical path is two cross-engine hops: PE score matmuls -> ACT exp
-> PE attn@V matmuls. The cache update at `position` never touches the
per-pair path: the stale K contribution is zeroed inside the exp via a
per-partition bias tile (-1e30 at partition pos%128, applied to kv-chunk
pos//128 only), and the new token's attention weight anew = exp(scale *
sum(q*k_new)) plus its V contribution anew*v_new are computed once, batched,
right after the projections. The per-head epilogue only sums partials
(+anew), broadcasts 1/Z, applies it, and runs o_proj; head 0's epilogue and
output DMA hide under the stream at pair 15.

Precision: cache matmuls and projections in bf16 (PE native rate) with fp32
PSUM accumulation; softmax sums, normalization and the new-token V term in
fp32. Measured vs fp32 reference: rel err ~5e-3.
"""

import sys

for _p in ("/opt/trn_rl_repo", "/root/.axon_site/_ro/trn_rl_repo"):
    if _p not in sys.path:
        sys.path.append(_p)

import ml_dtypes
import numpy as np

import concourse.bacc as bacc
import concourse.mybir as mybir
import concourse.tile as tile
from concourse.bass_utils import run_bass_kernel_spmd

F32 = mybir.dt.float32
BF16 = mybir.dt.bfloat16

B = 16          # batch
D = 2048        # model dim
H_TOT = 16      # total heads
HD = 128        # head dim
KV = 4096       # cache length
KVC = KV // 128  # 32 kv chunks of 128
N_CORES = 8
H_LOC = H_TOT // N_CORES       # 2 heads per core
PAIRS = H_LOC * B              # 32 (b,h) pairs per core
HS = H_LOC * HD                # 256-channel slice per core
DC = D // 128                  # 16 contraction chunks for projections

CH = 2                         # pairs per cache chunk (DMA granularity)
NCH = PAIRS // CH              # 16 chunks
CBUF = 4                       # cache chunks in flight per stream

# Matches reference: scale = 1.0 / np.sqrt(head_dim).astype(np.float32)
SCALE = float(1.0 / np.sqrt(float(HD)).astype(np.float32))

CDT = BF16
LAST_RESULT = None  # BassKernelResults of the most recent run (for profiling)


def build_kernel(position):
    """Trace the per-core Bass kernel. `position` is baked in as a constant."""
    pc, pi = position // 128, position % 128
    assert 0 <= position < KV

    nc = bacc.Bacc("TRN2", target_bir_lowering=False, debug=False)

    xT = nc.dram_tensor("xT", [128, DC, B], CDT, kind="ExternalInput").ap()
    wqT = nc.dram_tensor("wqT", [128, DC, HS], CDT, kind="ExternalInput").ap()
    wkT = nc.dram_tensor("wkT", [128, DC, HS], CDT, kind="ExternalInput").ap()
    wvT = nc.dram_tensor("wvT", [128, DC, HS], CDT, kind="ExternalInput").ap()
    woT = nc.dram_tensor("woT", [128, H_LOC, D], CDT, kind="ExternalInput").ap()
    cache = nc.dram_tensor(
        "cache", [128, NCH, 2, CH * KV], CDT, kind="ExternalInput"
    ).ap()
    yT = nc.dram_tensor("yT", [128, H_LOC, DC, B], F32, kind="ExternalOutput").ap()

    with tile.TileContext(nc) as tc:
        with (
            tc.tile_pool(name="wpool", bufs=1) as wpool,
            tc.tile_pool(name="spool", bufs=1) as spool,
            tc.tile_pool(name="kpool", bufs=CBUF) as kpool,
            tc.tile_pool(name="vpool", bufs=CBUF) as vpool,
            tc.tile_pool(name="ps_sc", bufs=3, space="PSUM") as ps_sc,
            tc.tile_pool(name="ps_one", bufs=1, space="PSUM") as ps_one,
        ):
            # ---- weights & x first on the sync (SP) HWDGE ring: they gate
            # the projections -> pair 0. wo rides the gpsimd queue (needed
            # only at pair 15). The SP engine issues nothing but DMAs, so a
            # pool-slot wait there never stalls compute. ----
            xT_sb = wpool.tile([128, DC, B], CDT)
            nc.sync.dma_start(xT_sb[:], xT)
            wq_sb = wpool.tile([128, DC, HS], CDT)
            nc.sync.dma_start(wq_sb[:], wqT)
            wk_sb = wpool.tile([128, DC, HS], CDT)
            nc.sync.dma_start(wk_sb[:], wkT)
            wv_sb = wpool.tile([128, DC, HS], CDT)
            nc.sync.dma_start(wv_sb[:], wvT)
            wo_sb = wpool.tile([128, H_LOC, D], CDT)
            nc.gpsimd.dma_start(wo_sb[:], woT)

            # ---- cache chunk stream, all on the sync ring ----
            kts, vts = {}, {}

            def issue_chunk(c):
                kt = kpool.tile([128, CH * KV], CDT, tag="kt", name="kt")
                nc.sync.dma_start(kt[:], cache[:, c, 0])
                kts[c] = kt
                vt = vpool.tile([128, CH * KV], CDT, tag="vt", name="vt")
                nc.sync.dma_start(vt[:], cache[:, c, 1])
                vts[c] = vt

            issue_chunk(0)
            issue_chunk(1)
            issue_chunk(2)

            # ---- constants ----
            ones_col = spool.tile([128, 1], F32)
            nc.vector.memset(ones_col[:], 1.0)
            ones_row = spool.tile([1, 128], F32)
            nc.vector.memset(ones_row[:], 1.0)
            # mbias: -1e30 at partition pi, 0 elsewhere (zeroes the stale K
            # contribution inside the exp of kv-chunk pc)
            negc = spool.tile([128, 1], F32)
            nc.vector.memset(negc[:], -1e30)
            mbias = spool.tile([128, 1], F32)
            nc.gpsimd.affine_select(
                mbias[:], negc[:], pattern=[[0, 1]],
                compare_op=mybir.AluOpType.is_equal, fill=0.0,
                base=-pi, channel_multiplier=1,
            )

            # ---- q/k/v projections -> (128 hd, 32 pair) columns ----
            qT_sb = spool.tile([128, PAIRS], CDT)
            kn_sb = spool.tile([128, PAIRS], CDT)
            vn_sb = spool.tile([128, PAIRS], F32)  # new-v term applied in fp32
            for w_sb, out_sb, ptag in (
                (wq_sb, qT_sb, "pj_a"),
                (wk_sb, kn_sb, "pj_b"),
                (wv_sb, vn_sb, "pj_a"),
            ):
                pj = ps_one.tile([128, PAIRS], F32, tag=ptag, name="pj")
                for h in range(H_LOC):
                    for c in range(DC):
                        nc.tensor.matmul(
                            pj[:, 16 * h : 16 * (h + 1)],
                            w_sb[:, c, 128 * h : 128 * (h + 1)],
                            xT_sb[:, c, :],
                            start=(c == 0),
                            stop=(c == DC - 1),
                        )
                nc.vector.tensor_copy(out_sb[:], pj[:])

            # ---- batched new-token terms (off the per-pair path):
            # anew = exp(scale * colsum(q .* k_new));  t1 = anew_bc .* v_new ----
            qkn = spool.tile([128, PAIRS], F32)
            nc.vector.tensor_tensor(qkn[:], qT_sb[:], kn_sb[:], mybir.AluOpType.mult)
            snew_ps = ps_one.tile([1, PAIRS], F32, tag="pj_b", name="snew")
            nc.tensor.matmul(snew_ps[:], ones_col[:], qkn[:], start=True, stop=True)
            anew_sb = spool.tile([1, PAIRS], F32)
            nc.scalar.activation(
                anew_sb[:], snew_ps[:], mybir.ActivationFunctionType.Exp, scale=SCALE
            )
            ab_ps = ps_one.tile([128, PAIRS], F32, tag="pj_b", name="ab_ps")
            nc.tensor.matmul(ab_ps[:], ones_row[:], anew_sb[:], start=True, stop=True)
            anew_bc = spool.tile([128, PAIRS], F32)
            nc.scalar.copy(anew_bc[:], ab_ps[:])
            t1_sb = spool.tile([128, PAIRS], F32)
            nc.vector.tensor_tensor(
                t1_sb[:], anew_bc[:], vn_sb[:], mybir.AluOpType.mult
            )

            # ---- per-pair state ----
            attn_sb = spool.tile([128, PAIRS * KVC], CDT)
            pA = spool.tile([128, PAIRS], F32)   # exp row-sums, kv chunks < pc
            pB = spool.tile([128, PAIRS], F32)   # exp row-sums, masked chunk pc
            pC = spool.tile([128, PAIRS], F32)   # exp row-sums, kv chunks > pc
            outU = ps_one.tile([128, PAIRS], F32, tag="outU")
            attout = spool.tile([128, PAIRS], CDT)
            yt_ps = [
                ps_one.tile([128, DC, B], F32, tag="yt0", name="yt0"),
                ps_one.tile([128, DC, B], F32, tag="yt1", name="yt1"),
            ]

            def front(p):
                c, i = p // CH, p % CH
                kt = kts[c]
                sc = ps_sc.tile([128, KVC], F32, tag="sc", name="sc")
                for j in range(KVC):
                    nc.tensor.matmul(
                        sc[:, j : j + 1],
                        kt[:, KV * i + 128 * j : KV * i + 128 * (j + 1)],
                        qT_sb[:, p : p + 1],
                        start=True,
                        stop=True,
                    )
                ab = attn_sb[:, KVC * p : KVC * (p + 1)]
                if pc > 0:
                    nc.scalar.activation(
                        ab[:, 0:pc], sc[:, 0:pc],
                        mybir.ActivationFunctionType.Exp,
                        scale=SCALE, accum_out=pA[:, p : p + 1],
                    )
                nc.scalar.activation(
                    ab[:, pc : pc + 1], sc[:, pc : pc + 1],
                    mybir.ActivationFunctionType.Exp,
                    scale=SCALE, bias=mbias[:], accum_out=pB[:, p : p + 1],
                )
                if pc < KVC - 1:
                    nc.scalar.activation(
                        ab[:, pc + 1 :], sc[:, pc + 1 :],
                        mybir.ActivationFunctionType.Exp,
                        scale=SCALE, accum_out=pC[:, p : p + 1],
                    )

            def back(p):
                c, i = p // CH, p % CH
                vt = vts[c]
                ab = attn_sb[:, KVC * p : KVC * (p + 1)]
                for j in range(KVC):
                    nc.tensor.matmul(
                        outU[:, p : p + 1],
                        vt[:, KV * i + 128 * j : KV * i + 128 * (j + 1)],
                        ab[:, j : j + 1],
                        start=(j == 0),
                        stop=(j == KVC - 1),
                    )

            recips = {}

            def epi_a(h):
                # Z per pair: sum of exp row-sums (+ the new token's weight),
                # then 1/Z. Split from epi_b so the Vector reciprocal overlaps
                # with the next fronts on PE.
                cs = slice(16 * h, 16 * (h + 1))
                es = ps_one.tile([1, 16], F32, tag="pj_a", name="es")
                nc.tensor.matmul(es[:], ones_col[:], pA[:, cs], start=True, stop=False)
                if pc < KVC - 1:
                    nc.tensor.matmul(
                        es[:], ones_col[:], pC[:, cs], start=False, stop=False
                    )
                nc.tensor.matmul(es[:], ones_col[:], pB[:, cs], start=False, stop=False)
                nc.tensor.matmul(
                    es[:], ones_row[:, :1], anew_sb[:, cs], start=False, stop=True
                )
                recip_h = spool.tile([1, 16], F32, tag=f"recip{h}", name="recip_h")
                nc.vector.reciprocal(recip_h[:], es[:])
                recips[h] = recip_h

            def epi_b(h):
                cs = slice(16 * h, 16 * (h + 1))
                rb = ps_one.tile([128, 16], F32, tag="pj_b", name="rb")
                nc.tensor.matmul(rb[:], ones_row[:], recips[h][:], start=True, stop=True)
                recip_bc = spool.tile([128, 16], F32, tag=f"rbc{h}", name="recip_bc")
                nc.vector.tensor_copy(recip_bc[:], rb[:])
                t2 = spool.tile([128, 16], F32, tag=f"t2{h}", name="t2")
                nc.vector.tensor_tensor(
                    t2[:], outU[:, cs], t1_sb[:, cs], mybir.AluOpType.add
                )
                nc.vector.tensor_tensor(
                    attout[:, cs], t2[:], recip_bc[:], mybir.AluOpType.mult
                )
                for dc in range(DC):
                    nc.tensor.matmul(
                        yt_ps[h][:, dc, :],
                        wo_sb[:, h, 128 * dc : 128 * (dc + 1)],
                        attout[:, cs],
                        start=True,
                        stop=True,
                    )
                yt_sb = spool.tile([128, DC, B], F32, tag=f"yt_sb{h}", name="yt_sb")
                nc.vector.tensor_copy(yt_sb[:], yt_ps[h][:])
                return yt_sb

            # software-pipelined: chunk c's fronts run while chunk c-1's backs
            # consume, DMA for chunk c+3 issues ahead (CBUF=4 in flight)
            for c in range(NCH):
                if c + 3 < NCH:
                    issue_chunk(c + 3)
                front(CH * c)
                front(CH * c + 1)
                if c == 9:
                    yt_sb0 = epi_b(0)
                if c > 0:
                    back(CH * (c - 1))
                    back(CH * (c - 1) + 1)
                if c == 8:
                    epi_a(0)  # pairs 0-15 (head 0) all finished at c==8
            back(PAIRS - 2)
            back(PAIRS - 1)
            # yt0 rides the sync FIFO right behind the last cache chunk
            nc.sync.dma_start(yT[:, 0], yt_sb0[:])
            epi_a(H_LOC - 1)
            yt_sb1 = epi_b(H_LOC - 1)
            nc.sync.dma_start(yT[:, 1], yt_sb1[:])

    nc.compile()
    return nc


def shard_inputs(x, Wq, Wk, Wv, Wo, k_cache, v_cache):
    """Build per-core input maps (head-sharded)."""
    cdt = ml_dtypes.bfloat16

    def sb_layout(a2d, inner):
        # (K*128, inner-layout...) -> (128, K, ...) contiguous per partition
        d0 = a2d.shape[0]
        return np.ascontiguousarray(
            a2d.reshape(d0 // 128, 128, a2d.shape[1]).transpose(1, 0, 2)
        ).astype(cdt)

    x2 = np.ascontiguousarray(np.asarray(x, dtype=np.float32).reshape(B, D))
    xT_full = sb_layout(np.ascontiguousarray(x2.T), B)        # (128, DC, B)

    # K: (hd, H, B, KV) ; V: (kv%128, H, B, KVC, hd) partition-swizzled
    kT_all = np.asarray(k_cache, dtype=np.float32).transpose(3, 1, 0, 2).astype(cdt)
    v_all = (
        np.asarray(v_cache, dtype=np.float32)
        .reshape(B, H_TOT, KVC, 128, HD)
        .transpose(3, 1, 0, 2, 4)
        .astype(cdt)
    )
    Wq = np.asarray(Wq, dtype=np.float32)
    Wk = np.asarray(Wk, dtype=np.float32)
    Wv = np.asarray(Wv, dtype=np.float32)
    Wo = np.asarray(Wo, dtype=np.float32)

    in_maps = []
    for c in range(N_CORES):
        r0, r1 = HS * c, HS * (c + 1)
        h0, h1 = H_LOC * c, H_LOC * (c + 1)
        ktr = kT_all[:, h0:h1].reshape(128, NCH, CH * KV)
        vtr = v_all[:, h0:h1].reshape(128, NCH, CH * KV)
        cache = np.ascontiguousarray(np.stack([ktr, vtr], axis=2))
        in_maps.append(
            {
                "xT": xT_full,
                "wqT": sb_layout(Wq[r0:r1].T, HS),
                "wkT": sb_layout(Wk[r0:r1].T, HS),
                "wvT": sb_layout(Wv[r0:r1].T, HS),
                "woT": sb_layout(Wo[:, r0:r1].T, D),
                "cache": cache,
            }
        )
    return in_maps


_NC_CACHE = {}


def kernel(x, Wq, Wk, Wv, Wo, k_cache, v_cache, position):
    global LAST_RESULT
    pos = int(position)
    nc = _NC_CACHE.get(pos)
    if nc is None:
        nc = _NC_CACHE[pos] = build_kernel(pos)
    in_maps = shard_inputs(x, Wq, Wk, Wv, Wo, k_cache, v_cache)
    res = run_bass_kernel_spmd(nc, in_maps, core_ids=list(range(N_CORES)))
    LAST_RESULT = res
    out = np.zeros((128, DC, B), dtype=np.float32)
    for c in range(N_CORES):
        out += res.results[c]["yT"].sum(axis=1)
    y2 = out.transpose(1, 0, 2).reshape(D, B)
    return np.ascontiguousarray(y2.T).reshape(B, 1, D)
